# revision 31
# baseline (speedup 1.0000x reference)
"""Trainium2 Bass kernel for nn_ButterflyFilter.

The reference chain (pad -> butterfly FFT -> ramp filter in bit-reversed
order -> butterfly IFFT -> Re[:512]) is linear in x, so it is one real
512x512 operator W = Re(A)[:512, :512] with A circulant. W is an exactly
symmetric Toeplitz matrix W[o, i] = g[o - i] with g the FBP ramp kernel
(g[0] = 1/2, g[odd d] = -2/(pi d)^2, g[even d] = 0), which decays like
1/d^2: a 64-wide staircase band changes the result by ~1.6e-4 relative;
with bf16 operands and output store the total is ~2.6e-3 (measured),
7x under the 2e-2 gate.

Banded + Toeplitz => each 128-row output chunk needs TWO input chunks on
a 64-shifted grid:
  out[128o : 128o+128] = Ga @ c_o + Gb @ c_{o+1},
  c_j = x rows [128j - 64, 128j + 64)   (zero-padded at the ends)
with the same two 128x128 stationaries for every o: 8 matmuls per
(b, c) tile, 16 per core (2 tiles/core, 8 cores), 64 KiB of operator.

Schedule facts this implementation is built around (from NTFF traces):
  - The DMA fabric is ~270 GB/s per core AGGREGATE across queues, so
    queues are specialized: Sync carries the 5 input pieces in exact PE
    consumption order; Scalar's queue carries outputs. Competing input
    streams starve the PE mid-stream.
  - A dma_start costs ~0.6-0.7 us of descriptor-gen on the issuing
    engine: inputs are fused into 5 pieces (64K + 256K/384K per tile)
    with 1-3 KiB partition lines.
  - Concurrently in-flight DMAs must not share a semaphore (their 16
    completion increments interleave out of order): one per piece.
  - PSUM->SBUF bf16 drains: DVE tensor_copy signals with then_inc
    directly (proven safe); ACT activation-copies are kept OFF the
    critical tail and signal via an explicit pipeline drain.
  - ~6 warm-up matmuls on garbage SBUF bridge program start to the
    first piece's arrival so the HAM clock ramp (1.2 -> 2.4 GHz after
    ~3.4 us of sustained PE activity) completes before the real stream;
    any PE idle gap resets the ramp credit.
"""

import os
import sys
import types
from contextlib import ExitStack

import numpy as np

import concourse.bass as bass
import concourse.mybir as mybir
from concourse.bass_utils import run_bass_kernel_spmd


def _ensure_axon_hooks():
    # concourse.bass_utils imports antenv.axon_hooks on the trace path; some
    # images lack that module. Provide a no-op holder so a BASS_TRACE env set
    # by the caller can't crash the run.
    try:
        import antenv.axon_hooks  # noqa: F401
    except Exception:
        m = types.ModuleType("antenv.axon_hooks")
        m._h = None
        m.set_axon_ntff_profile_hook = lambda h: setattr(m, "_h", h)
        m.get_axon_ntff_profile_hook = lambda: m._h
        sys.modules["antenv.axon_hooks"] = m


_ensure_axon_hooks()

N_CORES = 8
S = 512          # row length and angle count (moving dim)
NF = 1024        # padded length inside the reference
P = 128
H = 64           # chunk-grid shift
OC = 4           # output row chunks per tile
BC_PER_CORE = 2
N_WARM = int(os.environ.get("BUTTERFLY_NWARM", "9"))

last_exec_time_ns = None
last_results = None


def _butterfly_np(tw, x, increasing):
    B, n = x.shape
    m = tw.shape[0]
    order = range(m) if increasing else range(m - 1, -1, -1)
    for idx in order:
        s = 1 << idx
        t = tw[idx].reshape(n // (2 * s), s, 2, 2)
        xr = x.reshape(B, n // (2 * s), 2, s)
        x = np.einsum('gjik,bgkj->bgij', t, xr).reshape(B, n)
    return x


def _compose_w(twiddle_fft, twiddle_ifft, fourier_filter_br):
    """Fold twiddles+filter into the dense operator W[o, i] (512x512 f64)."""
    tw_fft = np.asarray(twiddle_fft, dtype=np.float64)
    tw_ifft = np.asarray(twiddle_ifft, dtype=np.float64)
    filt = np.asarray(fourier_filter_br, dtype=np.float64)
    tf = tw_fft[0, ..., 0] + 1j * tw_fft[0, ..., 1]
    ti = tw_ifft[0, ..., 0] + 1j * tw_ifft[0, ..., 1]
    X = np.eye(NF, dtype=np.complex128)
    X = _butterfly_np(tf, X, increasing=False)
    X = X * filt[None, :]
    X = _butterfly_np(ti, X, increasing=True)
    return np.real(X[:S, :S]).T.copy()


def _band_stationaries(W):
    """lhsT operands: lhsT_a[i', o'] = g[o'-i'+64], lhsT_b = g[o'-i'-64]."""
    g = W[:, 0]  # g[|d|]; W is symmetric Toeplitz to ~3e-8
    D = np.arange(P)[None, :] - np.arange(P)[:, None]  # D[i', o'] = o' - i'
    return g[np.abs(D + H)], g[np.abs(D - H)]


def _build_nc():
    bf16 = mybir.dt.bfloat16
    f32 = mybir.dt.float32

    nc = bass.Bass()
    # Input pieces in Sync-queue (= PE consumption) order, packed for fat
    # partition lines (4 KiB lines move ~257 GB/s vs ~190 at 2 KiB). The
    # operator piece goes alone first (64 KiB) so the stream starts early;
    # the c4 chunks ride a later piece (first needed by matmul #8).
    #   p0 = (128, 256)  [Ga | Gb]
    #   p1 = (128, 2048) [c0|c1|c2|c3] tile0     (4 KiB lines)
    #   pc4 = (128, 1024) [c4_t0 | c4_t1]
    #   p2 = (128, 1024) [c0|c1] tile1, p3 = (128, 1024) [c2|c3] tile1
    p0 = nc.declare_dram_parameter("p0", [P, 2 * P], bf16, isOutput=False)
    p1 = nc.declare_dram_parameter("p1", [P, 4 * S], bf16, isOutput=False)
    p2 = nc.declare_dram_parameter("p2", [P, 6 * S], bf16, isOutput=False)
    out0 = nc.declare_dram_parameter("out0", [P, OC * S], bf16, isOutput=True)
    out1 = nc.declare_dram_parameter("out1", [P, OC * S], bf16, isOutput=True)

    with ExitStack() as ctx:
        w_sb = ctx.enter_context(nc.sbuf_tensor("w_sb", [P, 2 * P], bf16))
        p1_sb = ctx.enter_context(nc.sbuf_tensor("p1_sb", [P, 4 * S], bf16))
        p2_sb = ctx.enter_context(nc.sbuf_tensor("p2_sb", [P, 6 * S], bf16))
        warm_sb = ctx.enter_context(nc.sbuf_tensor("warm_sb", [P, P + S], bf16))
        o_sb = [
            ctx.enter_context(nc.sbuf_tensor(f"o_sb{t}", [P, OC * S], bf16))
            for t in range(BC_PER_CORE)
        ]
        accs = [
            ctx.enter_context(nc.psum_tensor(f"acc{g}", [P, S], f32))
            for g in range(BC_PER_CORE * OC)
        ]
        s_i = [ctx.enter_context(nc.semaphore(f"s_i{j}")) for j in range(3)]
        s_pe = ctx.enter_context(nc.semaphore("s_pe"))
        s_cl = ctx.enter_context(nc.semaphore("s_cl"))   # DVE copies
        s_cr = ctx.enter_context(nc.semaphore("s_cr"))   # ACT copies
        s_out = ctx.enter_context(nc.semaphore("s_out"))
        block = ctx.enter_context(nc.Block())

        ga = w_sb[:, 0:P]
        gb = w_sb[:, P:2 * P]
        cs = [
            [
                p1_sb[:, bass.ts(j, S)] for j in range(4)
            ] + [p2_sb[:, bass.ts(0, S)]],
            [
                p2_sb[:, bass.ts(2, S)], p2_sb[:, bass.ts(3, S)],
                p2_sb[:, bass.ts(4, S)], p2_sb[:, bass.ts(5, S)],
                p2_sb[:, bass.ts(1, S)],
            ],
        ]

        @block.sync
        def _(sync):
            sync.dma_start(w_sb[:], p0[:]).then_inc(s_i[0], 16)
            sync.dma_start(p1_sb[:], p1[:]).then_inc(s_i[1], 16)
            sync.dma_start(p2_sb[:], p2[:]).then_inc(s_i[2], 16)
            # tile1 whole output (4 KiB lines move ~355 GB/s) once all of
            # tile1 is drained: DVE copies g4,g6,g7 (#3..#5) and ACT g5.
            sync.wait_ge(s_cl, 5)
            sync.wait_ge(s_cr, 3)
            sync.dma_start(out1[:], o_sb[1][:]).then_inc(s_out, 16)
            sync.wait_ge(s_out, 2 * 16)

        @block.tensor
        def _(tensor):
            # Warm-ups bridge program start to the first piece's arrival
            # (a PE idle gap before the HAM grant resets the ramp credit).
            # K=4 stationaries keep their SBUF traffic ~nil so the input
            # DMA stream isn't starved of SBUF write bandwidth.
            for _ in range(N_WARM):
                nc.tensor.matmul(
                    accs[-1][:], warm_sb[0:4, :P], warm_sb[0:4, P:],
                    start=True, stop=True,
                )
            tensor.wait_ge(s_i[0], 16)
            for t in range(BC_PER_CORE):
                a = OC * t
                c = cs[t]

                def mm(g, w_ap, c_ap, start, stop):
                    m = nc.tensor.matmul(
                        accs[g][:], w_ap, c_ap, start=start, stop=stop
                    )
                    if stop:
                        m.then_inc(s_pe, 1)

                # Ga/Gb alternated so a group closes every 2nd matmul — the
                # copy engines start draining as early as possible.
                if t == 0:
                    tensor.wait_ge(s_i[1], 16)
                    mm(a + 0, ga, c[0], True, False)
                    mm(a + 0, gb, c[1], False, True)
                    mm(a + 1, ga, c[1], True, False)
                    mm(a + 1, gb, c[2], False, True)
                    mm(a + 2, ga, c[2], True, False)
                    mm(a + 2, gb, c[3], False, True)
                    mm(a + 3, ga, c[3], True, False)
                    tensor.wait_ge(s_i[2], 16)
                    mm(a + 3, gb, c[4], False, True)
                else:
                    mm(a + 0, ga, c[0], True, False)
                    mm(a + 0, gb, c[1], False, True)
                    mm(a + 1, ga, c[1], True, False)
                    mm(a + 1, gb, c[2], False, True)
                    mm(a + 2, ga, c[2], True, False)
                    mm(a + 2, gb, c[3], False, True)
                    mm(a + 3, ga, c[3], True, False)
                    mm(a + 3, gb, c[4], False, True)

        @block.vector
        def _(vector):
            # DVE drains g0, g2 and the tail-critical g4, g6, g7; then_inc
            # rides the copy itself (v3.1-proven safe for DVE).
            for g in (0, 2, 4, 6, 7):
                t, o = divmod(g, OC)
                vector.wait_ge(s_pe, g + 1)
                nc.vector.tensor_copy(
                    o_sb[t][:, bass.ts(o, S)], accs[g][:]
                ).then_inc(s_cl, 1)

        @block.scalar
        def _(scalar):
            # ACT drains g1, g3, g5 back-to-back, signals once via a single
            # pipeline drain, then issues tile0's output on its queue.
            for g, (t, o) in ((1, (0, 1)), (3, (0, 3)), (5, (1, 1))):
                scalar.wait_ge(s_pe, g + 1)
                nc.scalar.copy(o_sb[t][:, bass.ts(o, S)], accs[g][:])
            scalar.drain().then_inc(s_cr, 3)
            scalar.wait_ge(s_cl, 2)
            scalar.dma_start(out0[:], o_sb[0][:]).then_inc(s_out, 16)

    return nc


def kernel(x, twiddle_fft, twiddle_ifft, fourier_filter_br):
    global last_exec_time_ns, last_results
    import ml_dtypes

    bf16 = ml_dtypes.bfloat16
    x = np.asarray(x, dtype=np.float32)
    b, c, s_len, a = x.shape
    assert (b, c, s_len, a) == (8, 2, S, S)

    W = _compose_w(twiddle_fft, twiddle_ifft, fourier_filter_br)
    la, lb = _band_stationaries(W)
    w_piece = np.ascontiguousarray(
        np.concatenate([la, lb], axis=1).astype(bf16)
    )
    x16 = x.reshape(b * c, S, S)
    zpad = np.zeros((H, S), dtype=bf16)

    in_maps = []
    for core in range(N_CORES):
        cks = []
        for t in range(BC_PER_CORE):
            xb = x16[BC_PER_CORE * core + t].astype(bf16)
            cks.append(
                [
                    np.concatenate([zpad, xb[0:H]], axis=0),
                    xb[H:H + P],
                    xb[H + P:H + 2 * P],
                    xb[H + 2 * P:H + 3 * P],
                    np.concatenate([xb[H + 3 * P:], zpad], axis=0),
                ]
            )
        cat = lambda parts: np.ascontiguousarray(np.concatenate(parts, axis=1))
        in_maps.append(
            {
                "p0": np.ascontiguousarray(w_piece),
                "p1": cat(cks[0][0:4]),
                "p2": cat([cks[0][4], cks[1][4]] + cks[1][0:4]),
            }
        )
    nc = _build_nc()
    trace = os.environ.get("BUTTERFLY_TRACE") == "1"
    res = run_bass_kernel_spmd(nc, in_maps, core_ids=list(range(N_CORES)), trace=trace)
    last_exec_time_ns = res.exec_time_ns
    last_results = res

    # outN[p, 512*o + a] = proj row 128*o + p of tile 2*core + N.
    q = np.empty((b * c, S, S), dtype=np.float32)
    for k in range(N_CORES):
        for t, name in enumerate(("out0", "out1")):
            y = np.asarray(res.results[k][name]).reshape(P, OC, S)
            q[BC_PER_CORE * k + t] = (
                y.transpose(1, 0, 2).reshape(S, S).astype(np.float32)
            )
    # q[bc, o, a] = proj.T[o, bc*512 + a]; reference output is
    # proj.T.reshape(b, c, s, a) — a pure reinterpret of the (512, 8192) buffer.
    out = q.transpose(1, 0, 2).reshape(S, b * c * a).reshape(b, c, s_len, a)
    return np.ascontiguousarray(out).astype(np.float32)


# revision 32
# speedup vs baseline: 1.1359x; 1.1359x over previous
"""Trainium2 Bass kernel for nn_ButterflyFilter.

The reference chain (pad -> butterfly FFT -> ramp filter in bit-reversed
order -> butterfly IFFT -> Re[:512]) is linear in x, so it is one real
512x512 operator W = Re(A)[:512, :512] with A circulant. W is an exactly
symmetric Toeplitz matrix W[o, i] = g[o - i] with g the FBP ramp kernel
(g[0] = 1/2, g[odd d] = -2/(pi d)^2, g[even d] = 0), which decays like
1/d^2: a 64-wide staircase band changes the result by ~1.6e-4 relative;
with bf16 operands and output store the total is ~2.6e-3 (measured),
7x under the 2e-2 gate.

Banded + Toeplitz => each 128-row output chunk needs TWO input chunks on
a 64-shifted grid:
  out[128o : 128o+128] = Ga @ c_o + Gb @ c_{o+1},
  c_j = x rows [128j - 64, 128j + 64)   (zero-padded at the ends)
with the same two 128x128 stationaries for every o: 8 matmuls per
(b, c) tile, 16 per core (2 tiles/core, 8 cores), 64 KiB of operator.

Schedule facts this implementation is built around (from NTFF traces):
  - The DMA fabric is ~270 GB/s per core AGGREGATE across queues, so
    queues are specialized: Sync carries the 5 input pieces in exact PE
    consumption order; Scalar's queue carries outputs. Competing input
    streams starve the PE mid-stream.
  - A dma_start costs ~0.6-0.7 us of descriptor-gen on the issuing
    engine: inputs are fused into 5 pieces (64K + 256K/384K per tile)
    with 1-3 KiB partition lines.
  - Concurrently in-flight DMAs must not share a semaphore (their 16
    completion increments interleave out of order): one per piece.
  - PSUM->SBUF bf16 drains: DVE tensor_copy signals with then_inc
    directly (proven safe); ACT activation-copies are kept OFF the
    critical tail and signal via an explicit pipeline drain.
  - ~6 warm-up matmuls on garbage SBUF bridge program start to the
    first piece's arrival so the HAM clock ramp (1.2 -> 2.4 GHz after
    ~3.4 us of sustained PE activity) completes before the real stream;
    any PE idle gap resets the ramp credit.
"""

import os
import sys
import types
from contextlib import ExitStack

import numpy as np

import concourse.bass as bass
import concourse.mybir as mybir
from concourse.bass_utils import run_bass_kernel_spmd


def _ensure_axon_hooks():
    # concourse.bass_utils imports antenv.axon_hooks on the trace path; some
    # images lack that module. Provide a no-op holder so a BASS_TRACE env set
    # by the caller can't crash the run.
    try:
        import antenv.axon_hooks  # noqa: F401
    except Exception:
        m = types.ModuleType("antenv.axon_hooks")
        m._h = None
        m.set_axon_ntff_profile_hook = lambda h: setattr(m, "_h", h)
        m.get_axon_ntff_profile_hook = lambda: m._h
        sys.modules["antenv.axon_hooks"] = m


_ensure_axon_hooks()

N_CORES = 8
S = 512          # row length and angle count (moving dim)
NF = 1024        # padded length inside the reference
P = 128
H = 64           # chunk-grid shift
OC = 4           # output row chunks per tile
BC_PER_CORE = 2
N_WARM = int(os.environ.get("BUTTERFLY_NWARM", "21"))

last_exec_time_ns = None
last_results = None


def _butterfly_np(tw, x, increasing):
    B, n = x.shape
    m = tw.shape[0]
    order = range(m) if increasing else range(m - 1, -1, -1)
    for idx in order:
        s = 1 << idx
        t = tw[idx].reshape(n // (2 * s), s, 2, 2)
        xr = x.reshape(B, n // (2 * s), 2, s)
        x = np.einsum('gjik,bgkj->bgij', t, xr).reshape(B, n)
    return x


def _compose_w(twiddle_fft, twiddle_ifft, fourier_filter_br):
    """Fold twiddles+filter into the dense operator W[o, i] (512x512 f64)."""
    tw_fft = np.asarray(twiddle_fft, dtype=np.float64)
    tw_ifft = np.asarray(twiddle_ifft, dtype=np.float64)
    filt = np.asarray(fourier_filter_br, dtype=np.float64)
    tf = tw_fft[0, ..., 0] + 1j * tw_fft[0, ..., 1]
    ti = tw_ifft[0, ..., 0] + 1j * tw_ifft[0, ..., 1]
    X = np.eye(NF, dtype=np.complex128)
    X = _butterfly_np(tf, X, increasing=False)
    X = X * filt[None, :]
    X = _butterfly_np(ti, X, increasing=True)
    return np.real(X[:S, :S]).T.copy()


def _band_stationaries(W):
    """lhsT operands: lhsT_a[i', o'] = g[o'-i'+64], lhsT_b = g[o'-i'-64]."""
    g = W[:, 0]  # g[|d|]; W is symmetric Toeplitz to ~3e-8
    D = np.arange(P)[None, :] - np.arange(P)[:, None]  # D[i', o'] = o' - i'
    return g[np.abs(D + H)], g[np.abs(D - H)]


def _build_nc():
    bf16 = mybir.dt.bfloat16
    f32 = mybir.dt.float32

    nc = bass.Bass()
    # Input pieces in Sync-queue (= PE consumption) order, packed for fat
    # partition lines (4 KiB lines move ~257 GB/s vs ~190 at 2 KiB). The
    # operator piece goes alone first (64 KiB) so the stream starts early;
    # the c4 chunks ride a later piece (first needed by matmul #8).
    #   p0 = (128, 256)  [Ga | Gb]
    #   p1 = (128, 2048) [c0|c1|c2|c3] tile0     (4 KiB lines)
    #   pc4 = (128, 1024) [c4_t0 | c4_t1]
    #   p2 = (128, 1024) [c0|c1] tile1, p3 = (128, 1024) [c2|c3] tile1
    p0 = nc.declare_dram_parameter("p0", [P, 2 * P], bf16, isOutput=False)
    p1 = nc.declare_dram_parameter("p1", [P, 4 * S], bf16, isOutput=False)
    p2 = nc.declare_dram_parameter("p2", [P, 4 * S], bf16, isOutput=False)
    p3 = nc.declare_dram_parameter("p3", [P, 2 * S], bf16, isOutput=False)
    out0 = nc.declare_dram_parameter("out0", [P, OC * S], bf16, isOutput=True)
    out1 = nc.declare_dram_parameter("out1", [P, OC * S], bf16, isOutput=True)

    with ExitStack() as ctx:
        w_sb = ctx.enter_context(nc.sbuf_tensor("w_sb", [P, 2 * P], bf16))
        p1_sb = ctx.enter_context(nc.sbuf_tensor("p1_sb", [P, 4 * S], bf16))
        p2_sb = ctx.enter_context(nc.sbuf_tensor("p2_sb", [P, 4 * S], bf16))
        p3_sb = ctx.enter_context(nc.sbuf_tensor("p3_sb", [P, 2 * S], bf16))
        warm_sb = ctx.enter_context(nc.sbuf_tensor("warm_sb", [P, P + S], bf16))
        o_sb = [
            ctx.enter_context(nc.sbuf_tensor(f"o_sb{t}", [P, OC * S], bf16))
            for t in range(BC_PER_CORE)
        ]
        accs = [
            ctx.enter_context(nc.psum_tensor(f"acc{g}", [P, S], f32))
            for g in range(BC_PER_CORE * OC)
        ]
        s_i = [ctx.enter_context(nc.semaphore(f"s_i{j}")) for j in range(4)]
        s_pe = ctx.enter_context(nc.semaphore("s_pe"))
        s_cl = ctx.enter_context(nc.semaphore("s_cl"))   # DVE copies
        s_cr = ctx.enter_context(nc.semaphore("s_cr"))   # ACT copies
        s_out = ctx.enter_context(nc.semaphore("s_out"))
        block = ctx.enter_context(nc.Block())

        ga = w_sb[:, 0:P]
        gb = w_sb[:, P:2 * P]
        cs = [
            [
                p1_sb[:, bass.ts(j, S)] for j in range(4)
            ] + [p2_sb[:, bass.ts(0, S)]],
            [
                p2_sb[:, bass.ts(2, S)], p2_sb[:, bass.ts(3, S)],
                p3_sb[:, bass.ts(0, S)], p3_sb[:, bass.ts(1, S)],
                p2_sb[:, bass.ts(1, S)],
            ],
        ]

        @block.sync
        def _(sync):
            sync.dma_start(w_sb[:], p0[:]).then_inc(s_i[0], 16)
            sync.dma_start(p1_sb[:], p1[:]).then_inc(s_i[1], 16)
            sync.dma_start(p2_sb[:], p2[:]).then_inc(s_i[2], 16)
            sync.dma_start(p3_sb[:], p3[:]).then_inc(s_i[3], 16)
            # tile1 whole output (4 KiB lines move ~355 GB/s) once all of
            # tile1 is drained: DVE copies g4,g6,g7 (#3..#5) and ACT g5.
            sync.wait_ge(s_cl, 5)
            sync.wait_ge(s_cr, 3)
            sync.dma_start(out1[:], o_sb[1][:]).then_inc(s_out, 16)
            sync.wait_ge(s_out, 2 * 16)

        @block.tensor
        def _(tensor):
            # Warm-ups bridge program start to the first piece's arrival.
            # They must be full-K: the HAM activity monitor weighs how much
            # of the array is busy, and low-K warm-ups fail to earn the
            # 2.4 GHz grant (measured: K=4 left the whole stream at 1.2).
            for _ in range(N_WARM):
                nc.tensor.matmul(
                    accs[-1][:, :2 * P], warm_sb[:, :P], warm_sb[:, P:P + 2 * P],
                    start=True, stop=True,
                )
            tensor.wait_ge(s_i[0], 16)
            for t in range(BC_PER_CORE):
                a = OC * t
                c = cs[t]

                def mm(g, w_ap, c_ap, start, stop):
                    m = nc.tensor.matmul(
                        accs[g][:], w_ap, c_ap, start=start, stop=stop
                    )
                    if stop:
                        m.then_inc(s_pe, 1)

                # Ga/Gb alternated so a group closes every 2nd matmul — the
                # copy engines start draining as early as possible.
                if t == 0:
                    tensor.wait_ge(s_i[1], 16)
                    mm(a + 0, ga, c[0], True, False)
                    mm(a + 0, gb, c[1], False, True)
                    mm(a + 1, ga, c[1], True, False)
                    mm(a + 1, gb, c[2], False, True)
                    mm(a + 2, ga, c[2], True, False)
                    mm(a + 2, gb, c[3], False, True)
                    mm(a + 3, ga, c[3], True, False)
                    tensor.wait_ge(s_i[2], 16)
                    mm(a + 3, gb, c[4], False, True)
                else:
                    mm(a + 0, ga, c[0], True, False)
                    mm(a + 0, gb, c[1], False, True)
                    mm(a + 1, ga, c[1], True, False)
                    tensor.wait_ge(s_i[3], 16)
                    mm(a + 1, gb, c[2], False, True)
                    mm(a + 2, ga, c[2], True, False)
                    mm(a + 2, gb, c[3], False, True)
                    mm(a + 3, ga, c[3], True, False)
                    mm(a + 3, gb, c[4], False, True)

        @block.vector
        def _(vector):
            # DVE drains g0, g2 and the tail-critical g4, g6, g7; then_inc
            # rides the copy itself (v3.1-proven safe for DVE).
            for g in (0, 2, 4, 6, 7):
                t, o = divmod(g, OC)
                vector.wait_ge(s_pe, g + 1)
                nc.vector.tensor_copy(
                    o_sb[t][:, bass.ts(o, S)], accs[g][:]
                ).then_inc(s_cl, 1)

        @block.scalar
        def _(scalar):
            # ACT drains g1, g3, g5 back-to-back, signals once via a single
            # pipeline drain, then issues tile0's output on its queue.
            for g, (t, o) in ((1, (0, 1)), (3, (0, 3)), (5, (1, 1))):
                scalar.wait_ge(s_pe, g + 1)
                nc.scalar.copy(o_sb[t][:, bass.ts(o, S)], accs[g][:])
            scalar.drain().then_inc(s_cr, 3)
            scalar.wait_ge(s_cl, 2)
            scalar.dma_start(out0[:], o_sb[0][:]).then_inc(s_out, 16)

    return nc


def kernel(x, twiddle_fft, twiddle_ifft, fourier_filter_br):
    global last_exec_time_ns, last_results
    import ml_dtypes

    bf16 = ml_dtypes.bfloat16
    x = np.asarray(x, dtype=np.float32)
    b, c, s_len, a = x.shape
    assert (b, c, s_len, a) == (8, 2, S, S)

    W = _compose_w(twiddle_fft, twiddle_ifft, fourier_filter_br)
    la, lb = _band_stationaries(W)
    w_piece = np.ascontiguousarray(
        np.concatenate([la, lb], axis=1).astype(bf16)
    )
    x16 = x.reshape(b * c, S, S)
    zpad = np.zeros((H, S), dtype=bf16)

    in_maps = []
    for core in range(N_CORES):
        cks = []
        for t in range(BC_PER_CORE):
            xb = x16[BC_PER_CORE * core + t].astype(bf16)
            cks.append(
                [
                    np.concatenate([zpad, xb[0:H]], axis=0),
                    xb[H:H + P],
                    xb[H + P:H + 2 * P],
                    xb[H + 2 * P:H + 3 * P],
                    np.concatenate([xb[H + 3 * P:], zpad], axis=0),
                ]
            )
        cat = lambda parts: np.ascontiguousarray(np.concatenate(parts, axis=1))
        in_maps.append(
            {
                "p0": np.ascontiguousarray(w_piece),
                "p1": cat(cks[0][0:4]),
                "p2": cat([cks[0][4], cks[1][4]] + cks[1][0:2]),
                "p3": cat(cks[1][2:4]),
            }
        )
    nc = _build_nc()
    trace = os.environ.get("BUTTERFLY_TRACE") == "1"
    res = run_bass_kernel_spmd(nc, in_maps, core_ids=list(range(N_CORES)), trace=trace)
    last_exec_time_ns = res.exec_time_ns
    last_results = res

    # outN[p, 512*o + a] = proj row 128*o + p of tile 2*core + N.
    q = np.empty((b * c, S, S), dtype=np.float32)
    for k in range(N_CORES):
        for t, name in enumerate(("out0", "out1")):
            y = np.asarray(res.results[k][name]).reshape(P, OC, S)
            q[BC_PER_CORE * k + t] = (
                y.transpose(1, 0, 2).reshape(S, S).astype(np.float32)
            )
    # q[bc, o, a] = proj.T[o, bc*512 + a]; reference output is
    # proj.T.reshape(b, c, s, a) — a pure reinterpret of the (512, 8192) buffer.
    out = q.transpose(1, 0, 2).reshape(S, b * c * a).reshape(b, c, s_len, a)
    return np.ascontiguousarray(out).astype(np.float32)
